# revision 4
# baseline (speedup 1.0000x reference)
"""Trainium2 Bass kernel for nn_Decoder: 2-layer LSTM decoder + log-softmax NLL.

Fully transposed recurrence, fp8 DoubleRow matmuls.

Per core (8-way data parallel over batch, BL=32 rows/core, zero collectives):
- All matmuls keep WEIGHTS stationary ([K=128, M=128] full PE tiles) and
  stream transposed activations (N=32 columns); no transposes in the loop.
- fp8e4m3 + MatmulPerfMode.DoubleRow (2 k-tiles per pass, 0.5 cycles/row) for
  the e/h gate matmuls and the vocab projection. zpre/bias/zlog injects bf16.
- zpre / bg1 injected with ONE 512-wide matmul each (identity stationary,
  host-packed moving operand) instead of 16x 32-wide.
- Gates: sigmoid(x) = 0.5 + 0.5*tanh(x/2) with input scales folded into
  host-prescaled weights -> one tanh ACT per layer; tanh+exp share the single
  `exp_and_others` ACT table (2 table loads total).
- ACT queue order per step: tA0, tA1, th0, th1 (tA1 hoisted ahead of th0 so
  the ACT engine isn't head-of-line blocked on the DVE u-chain).
- Cell state kept doubled (s = 2c); gate math = 3 scalar_tensor_tensor ops +
  1 tanh + 1 STT per layer (exact algebra; fixups folded into weights).
- transformh0 / zpre / zlog / target-z-part computed on host (z-only, tiny).
- PSUM accumulation: ONE start/stop pair per PSUM tile (slice-level start
  flags re-arm the zero region and wipe earlier slices on hardware).
"""

import numpy as np
import ml_dtypes

import concourse.tile as tile
import concourse.mybir as mybir
from concourse import bacc
from concourse import bass_utils

B, T, V, D, Z = 256, 40, 5000, 512, 128
NC = 8
BL = B // NC              # 32 batch rows per core
NT = T - 1                # 39 recurrent steps
COLS = NT * BL            # 1248 (t, b) columns per core
G = 4 * D                 # 2048 gate width
NTILE = (COLS + 127) // 128   # 10 vocab tiles (last has 96 cols)
NVG = (V + 1023) // 1024      # 5 vocab exp groups per tile (last 904)

bf16 = mybir.dt.bfloat16
f8 = mybir.dt.float8e4
f32 = mybir.dt.float32
AF = mybir.ActivationFunctionType
ALU = mybir.AluOpType
DR = mybir.MatmulPerfMode.DoubleRow

# cpk (bf16, 128 partitions): onescol | id128 | zprM | bg1M
CK_OC, CK_ID, CK_ZP, CK_BG = 0, 2, 2 + 128, 2 + 128 + 512
CKW = 2 + 128 + 2 * 512

VOC_START = 6      # first step allowed to pump vocab work
PRE_PACE = 1       # vocab items emitted before l0h (matmuls early)
POST_PACE = 1      # vocab items emitted after the tails (fills ACT idle)

_CACHE = {}


def _build():
    nc = bacc.Bacc("TRN2", target_bir_lowering=False, debug=False)

    def din(name, shape, dt):
        return nc.dram_tensor(name, shape, dt, kind="ExternalInput").ap()

    cpk_d = din("cpk", [128, CKW], bf16)      # packed bf16 consts
    cps_d = din("cps", [32, 128], bf16)       # selb
    zlog_d = din("zlog", [32, V], bf16)
    cp8_d = din("cp8", [128, 256], f8)        # h0i | h1i
    cpf_d = din("cpf", [128, 256], f32)       # s0i | s1i
    w0e_d = din("w0e", [128, 4 * G], f8)
    eT_d = din("eT", [128, 4, NT * BL], f8)
    w0h_d = din("w0h", [128, 4 * G], f8)
    w1_d = din("w1", [128, 8 * G], f8)
    wout_d = din("wout", [128, 4 * V], f8)
    wta_d = din("wta", [128, 4 * COLS], f8)
    out_d = nc.dram_tensor("out_sd", [128, 32], f32,
                           kind="ExternalOutput").ap()

    with tile.TileContext(nc) as tc:
        from contextlib import ExitStack
        with ExitStack() as ctx:
            const = ctx.enter_context(tc.tile_pool(name="const", bufs=1))
            wpool = ctx.enter_context(tc.tile_pool(name="w", bufs=1))
            # startup-critical DMA order: w0e+eT head (first e-matmuls),
            # small const packs, w0h, then everything else
            w0e = wpool.tile([128, 2, 2, G], f8)
            nc.sync.dma_start(w0e[:], w0e_d[:])
            eT = wpool.tile([128, 2, 2, NT * BL], f8)
            # head: first 2 steps' embeddings so slot 0/1 e-matmuls can start
            # before the bulk lands
            nc.sync.dma_start(eT[:, :, :, 0:2 * BL], eT_d[:, :, 0:2 * BL])
            cpk = const.tile([128, CKW], bf16, tag="cpk")
            nc.sync.dma_start(cpk[:], cpk_d[:])
            cp8 = const.tile([128, 2, 4, 32], f8, tag="cp8")
            nc.sync.dma_start(cp8[:], cp8_d[:])
            cpf = const.tile([128, 256], f32, tag="cpf")
            nc.sync.dma_start(cpf[:], cpf_d[:])
            w0h = wpool.tile([128, 2, 2, G], f8)
            nc.sync.dma_start(w0h[:], w0h_d[:])
            nc.sync.dma_start(eT[:, :, :, 2 * BL:NT * BL],
                              eT_d[:, :, 2 * BL:NT * BL])
            w1 = wpool.tile([128, 4, 2, G], f8)
            nc.sync.dma_start(w1[:], w1_d[:])
            cps = const.tile([32, 128], bf16, tag="cps")
            nc.sync.dma_start(cps[:], cps_d[:])
            zlogt = const.tile([32, V], bf16, tag="zlogt")
            nc.sync.dma_start(zlogt[:], zlog_d[:])
            wout = wpool.tile([128, 2, 2, V], f8)
            nc.sync.dma_start(wout[:], wout_d[:])
            wta = wpool.tile([128, 4, COLS], f8)
            nc.sync.dma_start(wta[:], wta_d[:])

            onescol = cpk[:, CK_OC:CK_OC + 2]
            id128 = cpk[:, CK_ID:CK_ID + 128]
            zprM = cpk[:, CK_ZP:CK_ZP + 512]
            bg1M = cpk[:, CK_BG:CK_BG + 512]
            selb = cps[0:32, 0:128]
            zlog = zlogt
            h0i = cp8[:, 0, :, :]
            h1i = cp8[:, 1, :, :]
            s0i = cpf[:, 0:128]
            s1i = cpf[:, 128:256]

            state = ctx.enter_context(tc.tile_pool(name="state", bufs=1))
            HT = state.tile([128, 4, COLS], f8)
            sd_all = state.tile([128, 32], f32, tag="sd_all")
            sums_all = sd_all[:, 0:16]
            dps_all = sd_all[:, 16:32]
            nc.vector.memset(sd_all[:], 1.0)

            sact = ctx.enter_context(tc.tile_pool(name="sact", bufs=6))
            sdve = ctx.enter_context(tc.tile_pool(name="sdve", bufs=6))
            sst = ctx.enter_context(tc.tile_pool(name="sst", bufs=6))
            sexp = ctx.enter_context(tc.tile_pool(name="sexp", bufs=6))
            gsum = ctx.enter_context(tc.tile_pool(name="gsum", bufs=4))
            pvoc_cm = tc.tile_pool(name="pvoc", bufs=2, space="PSUM")
            pvoc = pvoc_cm.__enter__()

            # ---------------- vocab + target-dot pump ---------------------
            gsums = {}
            vwork = []
            vpushed = 0
            pending_dve = []

            def emit_vgroup(j, vi):
                base = 128 * j
                mj = min(128, COLS - base)
                vg0 = 1024 * vi
                vgs = min(1024, V - vg0)
                pl = pvoc.tile([128, 1024], f32, tag="pl")
                # DR moving operand free size is 2*vs -> keep vs <= 256.
                # pl spans TWO psum banks: zero region is per-bank, so the
                # first matmul touching EACH bank carries start=True and the
                # last one stop=True.
                for q in range(0, vgs, 256):
                    v0 = vg0 + q
                    vs = min(256, V - v0)
                    qs = slice(q, q + vs)
                    for g in range(2):
                        nc.tensor.matmul(
                            pl[:mj, qs], HT[:, 2 * g:2 * g + 2, base:base + mj],
                            wout[:, g, :, v0:v0 + vs],
                            start=(q % 512 == 0 and g == 0), stop=False,
                            perf_mode=DR, skip_group_check=True)
                for half in range(0, vgs, 512):
                    v0 = vg0 + half
                    vs = min(512, V - v0)
                    nc.tensor.matmul(pl[:mj, half:half + vs], selb[:, 0:mj],
                                     zlog[:, v0:v0 + vs],
                                     start=False, stop=True,
                                     skip_group_check=True)
                es = sexp.tile([128, 1024], bf16, tag="es")
                if vi == 0 and j < NTILE - 1:
                    gsums[j] = gsum.tile([128, 8], f32, tag="gs",
                                         name=f"gs{j}")
                # last tile: partials go straight into spare sums_all cols
                # (host sums them) -- drops the final reduce from the
                # end-of-program critical path
                tgt = (sums_all[:mj, 9 + vi:10 + vi] if j == NTILE - 1
                       else gsums[j][:mj, vi:vi + 1])
                if vi % 2 == 0:
                    nc.scalar.activation(es[:mj, 0:vgs], pl[:mj, 0:vgs],
                                         AF.Exp, accum_out=tgt)
                else:
                    # alternate: sum on DVE (saves ACT read-accum aux),
                    # deferred past the chain-critical u-chain STTs
                    et = es
                    pending_dve.append(
                        lambda tgt=tgt, et=et, mj=mj, vgs=vgs:
                        nc.vector.tensor_reduce(
                            tgt, et[:mj, 0:vgs],
                            mybir.AxisListType.XYZW, ALU.add))
                    nc.scalar.activation(es[:mj, 0:vgs], pl[:mj, 0:vgs],
                                         AF.Exp)
                if vi == NVG - 1 and j < NTILE - 1:
                    gt = gsums[j]
                    pending_dve.append(
                        lambda gt=gt, mj=mj, j=j:
                        nc.vector.tensor_reduce(
                            sums_all[:mj, j:j + 1], gt[:mj, 0:NVG],
                            mybir.AxisListType.XYZW, ALU.add))

            def emit_wta(j):
                base = 128 * j
                mj = min(128, COLS - base)
                dps = pdot.tile([128, 2], f32, tag="dps")
                for c in range(4):
                    sc = sexp.tile([128, 128], bf16, tag="sc")
                    nc.vector.tensor_mul(sc[:, 0:mj],
                                         HT[:, c, base:base + mj],
                                         wta[:, c, base:base + mj])
                    nc.tensor.matmul(dps[:mj, 0:2], sc[:, 0:mj],
                                     onescol[:, 0:2],
                                     start=(c == 0), stop=(c == 3))
                nc.vector.tensor_copy(dps_all[:mj, j:j + 1], dps[:mj, 0:1])

            def vocab_pump(t_done, n):
                nonlocal vpushed
                while vpushed < NTILE and min(4 * vpushed + 4, NT - 1) <= t_done:
                    for vi in range(NVG):
                        vwork.append((vpushed, vi))
                    vwork.append((vpushed, -1))
                    vpushed += 1
                for _ in range(n):
                    if not vwork:
                        return
                    j, vi = vwork.pop(0)
                    if vi < 0:
                        emit_wta(j)
                    else:
                        emit_vgroup(j, vi)

            # ---------------- main recurrent loop --------------------------
            with tc.tile_pool(name="p0g", bufs=2, space="PSUM") as p0g, \
                 tc.tile_pool(name="p1g", bufs=1, space="PSUM") as p1g, \
                 tc.tile_pool(name="pdot", bufs=1, space="PSUM") as pdot:

                h0s_of = {-1: h0i}
                h1s_of = {-2: h1i, -1: h1i}
                s0_prev = s0i
                s1_prev = s1i

                for t in range(NT):
                    # l0 gate group: e-part + zpre (no recurrence deps)
                    g0 = p0g.tile([128, 512], f32, tag="g0")
                    for g in range(2):
                        for m in range(16):
                            ms = slice(32 * m, 32 * m + 32)
                            js = slice(128 * m, 128 * m + 128)
                            nc.tensor.matmul(
                                g0[:, ms], w0e[:, g, :, js],
                                eT[:, g, :, BL * t:BL * t + BL],
                                start=(m == 0 and g == 0), stop=False,
                                perf_mode=DR, skip_group_check=True)
                    nc.tensor.matmul(g0[:, 0:512], id128, zprM,
                                     start=False, stop=False,
                                     skip_group_check=True)

                    # l1(t-1) part A: h1(t-2) chunks + bias
                    if t > 0:
                        g1 = p1g.tile([128, 512], f32, tag="g1")
                        h1p = h1s_of[t - 2]
                        for m in range(16):
                            ms = slice(32 * m, 32 * m + 32)
                            js = slice(128 * m, 128 * m + 128)
                            for g in range(2):
                                nc.tensor.matmul(
                                    g1[:, ms], w1[:, g, :, js],
                                    h1p[:, 2 * g:2 * g + 2, :],
                                    start=(m == 0 and g == 0), stop=False,
                                    perf_mode=DR, skip_group_check=True)
                        nc.tensor.matmul(g1[:, 0:512], id128, bg1M,
                                         start=False, stop=False,
                                         skip_group_check=True)

                    # vocab filler: matmuls early so exp input is ready
                    if t >= VOC_START:
                        vocab_pump(t - 2, PRE_PACE)

                    # l0 h-part (closes g0) -- needs h0s(t-1)
                    h0p = h0s_of[t - 1]
                    for m in range(16):
                        ms = slice(32 * m, 32 * m + 32)
                        js = slice(128 * m, 128 * m + 128)
                        for g in range(2):
                            nc.tensor.matmul(
                                g0[:, ms], w0h[:, g, :, js],
                                h0p[:, 2 * g:2 * g + 2, :],
                                start=False,
                                stop=(m == 15 and g == 1),
                                perf_mode=DR, skip_group_check=True)

                    # l1(t-1) part B: h0(t-1) chunks (closes g1)
                    if t > 0:
                        for m in range(16):
                            ms = slice(32 * m, 32 * m + 32)
                            js = slice(128 * m, 128 * m + 128)
                            for g in range(2):
                                nc.tensor.matmul(
                                    g1[:, ms], w1[:, 2 + g, :, js],
                                    h0p[:, 2 * g:2 * g + 2, :],
                                    start=False,
                                    stop=(m == 15 and g == 1),
                                    perf_mode=DR, skip_group_check=True)

                    # ---- elementwise tails. ACT order: tA0, tA1, th0, th1
                    s01 = sst.tile([128, 256], f32, tag="s01")
                    tA0 = sact.tile([128, 512], bf16, tag="a0", name="tA0")
                    nc.scalar.activation(tA0[:], g0[:], AF.Tanh)
                    if t > 0:
                        tA1 = sact.tile([128, 512], bf16, tag="a1",
                                        name="tA1")
                        nc.scalar.activation(tA1[:], g1[:], AF.Tanh)
                    # DVE: layer-0 u-chain
                    u10 = sdve.tile([128, 128], f32, tag="u10", name="u10")
                    nc.vector.scalar_tensor_tensor(
                        u10[:], tA0[:, 0:128], 1.0, s0_prev, ALU.add, ALU.mult)
                    u20 = sdve.tile([128, 128], f32, tag="u20", name="u20")
                    nc.vector.scalar_tensor_tensor(
                        u20[:], tA0[:, 128:256], 1.0, tA0[:, 256:384],
                        ALU.add, ALU.mult)
                    nc.vector.scalar_tensor_tensor(
                        s01[:, 0:128], u10[:], 0.5, u20[:], ALU.mult, ALU.add)
                    th01 = sdve.tile([128, 256], bf16, tag="th01")
                    nc.scalar.activation(th01[:, 0:128], s01[:, 0:128],
                                         AF.Tanh, scale=0.5)
                    s0_prev = s01[:, 0:128]
                    if t > 0:
                        # layer-1 u-chain start (fits before th0 lands)
                        u11 = sdve.tile([128, 128], f32, tag="u11",
                                        name="u11")
                        nc.vector.scalar_tensor_tensor(
                            u11[:], tA1[:, 0:128], 1.0, s1_prev,
                            ALU.add, ALU.mult)
                    # h0 chain: hsn0 right after th0
                    hsn0 = sst.tile([128, 4, 32], f8, tag="h0s", name="hsn0")
                    nc.vector.scalar_tensor_tensor(
                        hsn0[:], tA0[:, 384:512], 1.0, th01[:, 0:128],
                        ALU.add, ALU.mult)
                    h0s_of[t] = hsn0
                    if t > 0:
                        u21 = sdve.tile([128, 128], f32, tag="u21",
                                        name="u21")
                        nc.vector.scalar_tensor_tensor(
                            u21[:], tA1[:, 128:256], 1.0, tA1[:, 256:384],
                            ALU.add, ALU.mult)
                        nc.vector.scalar_tensor_tensor(
                            s01[:, 128:256], u11[:], 0.5, u21[:],
                            ALU.mult, ALU.add)
                        nc.scalar.activation(th01[:, 128:256],
                                             s01[:, 128:256],
                                             AF.Tanh, scale=0.5)
                        hsn1 = sst.tile([128, 4, 32], f8, tag="h1s",
                                        name="hsn1")
                        nc.vector.scalar_tensor_tensor(
                            hsn1[:], tA1[:, 384:512], 1.0, th01[:, 128:256],
                            ALU.add, ALU.mult)
                        h1s_of[t - 1] = hsn1
                        s1_prev = s01[:, 128:256]
                        tw = t - 1
                        for c in range(4):
                            nc.gpsimd.tensor_add(
                                HT[:, c, 32 * tw:32 * tw + 32],
                                h0s_of[tw][:, c, :], hsn1[:, c, :])
                        del h1s_of[t - 3]
                        del h0s_of[t - 2]
                    # vocab filler: exp lands after th1 in the ACT queue
                    if t >= VOC_START:
                        vocab_pump(t - 2, POST_PACE)
                    for fn in pending_dve:
                        fn()
                    pending_dve.clear()

                # flush l1(NT-1)
                t = NT
                g1 = p1g.tile([128, 512], f32, tag="g1")
                h1p = h1s_of[t - 2]
                h0p = h0s_of[t - 1]
                for m in range(16):
                    ms = slice(32 * m, 32 * m + 32)
                    js = slice(128 * m, 128 * m + 128)
                    for g in range(2):
                        nc.tensor.matmul(g1[:, ms], w1[:, g, :, js],
                                         h1p[:, 2 * g:2 * g + 2, :],
                                         start=(m == 0 and g == 0), stop=False,
                                         perf_mode=DR, skip_group_check=True)
                nc.tensor.matmul(g1[:, 0:512], id128, bg1M,
                                 start=False, stop=False,
                                 skip_group_check=True)
                for m in range(16):
                    ms = slice(32 * m, 32 * m + 32)
                    js = slice(128 * m, 128 * m + 128)
                    for g in range(2):
                        nc.tensor.matmul(g1[:, ms], w1[:, 2 + g, :, js],
                                         h0p[:, 2 * g:2 * g + 2, :],
                                         start=False,
                                         stop=(m == 15 and g == 1),
                                         perf_mode=DR, skip_group_check=True)

                s01f = sst.tile([128, 256], f32, tag="s01")
                tA1f = sact.tile([128, 512], bf16, tag="a1", name="tA1f")
                nc.scalar.activation(tA1f[:], g1[:], AF.Tanh)
                u11f = sdve.tile([128, 128], f32, tag="u11", name="u11f")
                nc.vector.scalar_tensor_tensor(
                    u11f[:], tA1f[:, 0:128], 1.0, s1_prev, ALU.add, ALU.mult)
                u21f = sdve.tile([128, 128], f32, tag="u21", name="u21f")
                nc.vector.scalar_tensor_tensor(
                    u21f[:], tA1f[:, 128:256], 1.0, tA1f[:, 256:384],
                    ALU.add, ALU.mult)
                nc.vector.scalar_tensor_tensor(
                    s01f[:, 128:256], u11f[:], 0.5, u21f[:],
                    ALU.mult, ALU.add)
                th01f = sdve.tile([128, 256], bf16, tag="th01")
                nc.scalar.activation(th01f[:, 128:256], s01f[:, 128:256],
                                     AF.Tanh, scale=0.5)
                hsn1f = sst.tile([128, 4, 32], f8, tag="h1s", name="hsn1f")
                nc.vector.scalar_tensor_tensor(
                    hsn1f[:], tA1f[:, 384:512], 1.0, th01f[:, 128:256],
                    ALU.add, ALU.mult)
                tw = NT - 1
                for c in range(4):
                    nc.gpsimd.tensor_add(
                        HT[:, c, 32 * tw:32 * tw + 32],
                        h0s_of[tw][:, c, :], hsn1f[:, c, :])

                # drain remaining vocab + wta work, then finalize
                vocab_pump(NT - 1, len(vwork) + NVG + 2)
                for fn in pending_dve:
                    fn()
                pending_dve.clear()
                nc.sync.dma_start(out_d[:, :], sd_all[:, :])
            pvoc_cm.__exit__(None, None, None)

    nc.compile()
    return nc


def _prep_host(inputs):
    z = np.asarray(inputs["z"], np.float32)
    x = np.asarray(inputs["x"])
    emb = np.asarray(inputs["emb"], np.float32)
    Wg0 = np.asarray(inputs["Wg0"], np.float32)
    bg0 = np.asarray(inputs["bg0"], np.float32)
    Wg1 = np.asarray(inputs["Wg1"], np.float32)
    bg1 = np.asarray(inputs["bg1"], np.float32)
    Wout = np.asarray(inputs["Wout"], np.float32)
    bout = np.asarray(inputs["bout"], np.float32)
    tw1 = np.asarray(inputs["tw1"], np.float32)
    tb1 = np.asarray(inputs["tb1"], np.float32)
    tw2 = np.asarray(inputs["tw2"], np.float32)
    tb2 = np.asarray(inputs["tb2"], np.float32)

    bf = ml_dtypes.bfloat16
    f8h = ml_dtypes.float8_e4m3fn

    def permute_rows(W):
        # (i, f, o, cn) blocks -> (f, i, cn, o)
        return np.concatenate(
            [W[512:1024], W[0:512], W[1536:2048], W[1024:1536]], axis=0)

    rs = np.repeat([0.5, 0.5, 1.0, 0.5], 512).astype(np.float32)[:, None]

    def chunked(a, nch):
        # [128*nch, N] -> [128, nch*N] (chunk-major free layout)
        n = a.shape[1]
        return np.ascontiguousarray(
            a.reshape(nch, 128, n).transpose(1, 0, 2).reshape(128, nch * n))

    W0 = Wg0.reshape(G, D + Z + D)
    W0p = permute_rows(W0) * rs
    bg0p = (permute_rows(bg0.reshape(G, 1)) * rs)[:, 0]
    W1 = Wg1.reshape(G, 2 * D)
    W1p = permute_rows(W1) * rs * 0.5
    bg1p = (permute_rows(bg1.reshape(G, 1)) * rs)[:, 0]
    W0z_s = W0p[:, 1024:1152]

    shared = {
        # chunked() already yields the [q, chunk, ...] layout; DoubleRow just
        # reinterprets chunk index as (pair g, ktile p)
        "w0h": chunked(W0p[:, 0:512].T * 0.5, 4).astype(f8h),
        "w0e": chunked(W0p[:, 512:1024].T, 4).astype(f8h),
        "w1": chunked(W1p.T, 8).astype(f8h),
        "wout": chunked(Wout[:, 0:512].T * 0.5, 4).astype(f8h),
    }

    def injM(ZP):
        # [32 b, 2048 gate] -> [128 q, 512 (m,b)] moving layout:
        # injM[q, 32m+b] = ZP[b, 128m+q]
        return np.ascontiguousarray(
            ZP.T.reshape(16, 128, 32).transpose(1, 0, 2).reshape(128, 512))

    cps0 = np.tile(np.eye(32, dtype=bf), (1, 4))  # selb [32, 128]

    cpk_base = np.zeros((128, CKW), bf)
    cpk_base[:, CK_OC:CK_OC + 2] = 1.0
    cpk_base[:, CK_ID:CK_ID + 128] = np.eye(128, dtype=bf)
    cpk_base[:, CK_BG:CK_BG + 512] = injM(
        np.broadcast_to(bg1p, (32, G))).astype(bf)

    def packT(a):  # [32, 512] -> [128, 128] transposed chunk-packed
        return np.ascontiguousarray(
            a.T.reshape(4, 128, 32).transpose(1, 0, 2).reshape(128, 128))

    in_maps = []
    extra = []
    for cidx in range(NC):
        bs = slice(BL * cidx, BL * cidx + BL)
        z_c = z[bs]
        x_c = x[bs]
        xn = x_c[:, 1:T]

        m = dict(shared)
        cp8 = np.zeros((128, 256), f8h)
        cpf = np.zeros((128, 256), np.float32)
        for l in range(2):
            u = np.maximum(z_c @ tw1[l].T + tb1[l], 0.0)
            hh = np.tanh(u @ tw2[l].T + tb2[l])
            cp8[:, 128 * l:128 * l + 128] = packT(
                2.0 * hh[:, 0:512]).astype(f8h)
            cpf[:, 128 * l:128 * l + 128] = packT(2.0 * hh[:, 512:1024])
        m["cp8"] = cp8
        m["cpf"] = cpf

        cpk = cpk_base.copy()
        cpk[:, CK_ZP:CK_ZP + 512] = injM(
            (z_c @ W0z_s.T + bg0p).astype(np.float32)).astype(bf)
        m["cpk"] = cpk
        m["cps"] = cps0
        zlog_f = z_c @ Wout[:, 512:640].T + bout
        m["zlog"] = zlog_f.astype(bf)
        tdz = np.take_along_axis(zlog_f, xn, axis=1)
        extra.append(tdz.sum(axis=1))

        embx = emb[x_c[:, 0:NT]]
        m["eT"] = np.ascontiguousarray(
            embx.transpose(2, 1, 0).reshape(4, 128, NT * BL)
            .transpose(1, 0, 2).reshape(128, 4 * NT * BL)).astype(f8h)
        wrows = Wout[xn][:, :, 0:512] * 0.5
        m["wta"] = np.ascontiguousarray(
            wrows.transpose(2, 1, 0).reshape(4, 128, COLS)
            .transpose(1, 0, 2).reshape(128, 4 * COLS)).astype(f8h)
        in_maps.append(m)
    return in_maps, extra


def kernel(**inputs) -> np.ndarray:
    if "nc" not in _CACHE:
        _CACHE["nc"] = _build()
    nc = _CACHE["nc"]
    in_maps, extra = _prep_host(inputs)
    res = bass_utils.run_bass_kernel_spmd(nc, in_maps, core_ids=list(range(NC)))
    out = np.zeros((B, 1), np.float32)
    for cidx in range(NC):
        sd = np.array(res.results[cidx]["out_sd"])   # [128, 32]
        sd[:, NTILE - 1] = sd[:, 9:14].sum(axis=1)   # fold last tile partials
        lpc = sd[:, 16:16 + NTILE] - np.log(sd[:, 0:NTILE])
        lp = lpc.T.reshape(-1)[:COLS].reshape(NT, BL)
        out[BL * cidx:BL * cidx + BL, 0] = lp.sum(axis=0) + extra[cidx]
    return out


# revision 10
# speedup vs baseline: 1.0111x; 1.0111x over previous
"""Trainium2 Bass kernel for nn_Decoder: 2-layer LSTM decoder + log-softmax NLL.

Fully transposed recurrence, fp8 DoubleRow matmuls.

Per core (8-way data parallel over batch, BL=32 rows/core, zero collectives):
- All matmuls keep WEIGHTS stationary ([K=128, M=128] full PE tiles) and
  stream transposed activations (N=32 columns); no transposes in the loop.
- fp8e4m3 + MatmulPerfMode.DoubleRow (2 k-tiles per pass, 0.5 cycles/row) for
  the e/h gate matmuls and the vocab projection. zpre/bias/zlog injects bf16.
- zpre / bg1 injected with ONE 512-wide matmul each (identity stationary,
  host-packed moving operand) instead of 16x 32-wide.
- Gates: sigmoid(x) = 0.5 + 0.5*tanh(x/2) with input scales folded into
  host-prescaled weights -> one tanh ACT per layer; tanh+exp share the single
  `exp_and_others` ACT table (2 table loads total).
- ACT queue order per step: tA0, tA1, th0, th1 (tA1 hoisted ahead of th0 so
  the ACT engine isn't head-of-line blocked on the DVE u-chain).
- Cell state kept doubled (s = 2c); gate math = 3 scalar_tensor_tensor ops +
  1 tanh + 1 STT per layer (exact algebra; fixups folded into weights).
- transformh0 / zpre / zlog / target-z-part computed on host (z-only, tiny).
- PSUM accumulation: ONE start/stop pair per PSUM tile (slice-level start
  flags re-arm the zero region and wipe earlier slices on hardware).
"""

import numpy as np
import ml_dtypes

import concourse.tile as tile
import concourse.mybir as mybir
from concourse import bacc
from concourse import bass_utils

B, T, V, D, Z = 256, 40, 5000, 512, 128
NC = 8
BL = B // NC              # 32 batch rows per core
NT = T - 1                # 39 recurrent steps
COLS = NT * BL            # 1248 (t, b) columns per core
G = 4 * D                 # 2048 gate width
NTILE = (COLS + 127) // 128   # 10 vocab tiles (last has 96 cols)
NVG = (V + 1023) // 1024      # 5 vocab exp groups per tile (last 904)

bf16 = mybir.dt.bfloat16
f8 = mybir.dt.float8e4
f32 = mybir.dt.float32
AF = mybir.ActivationFunctionType
ALU = mybir.AluOpType
DR = mybir.MatmulPerfMode.DoubleRow

# cpk (bf16, 128 partitions): onescol | id128 | zprM | bg1M
CK_OC, CK_ID, CK_ZP, CK_BG = 0, 2, 2 + 128, 2 + 128 + 512
CKW = 2 + 128 + 2 * 512

VOC_START = 6      # first step allowed to pump vocab work
PRE_PACE = 1       # vocab items emitted before l0h (matmuls early)
POST_PACE = 1      # vocab items emitted after the tails (fills ACT idle)
VOFF = 300         # vocab priority penalty (~2 steps of instructions)
L1OFF = 60         # layer-1 tail priority penalty (~1 step of slack)

_CACHE = {}


def _build():
    nc = bacc.Bacc("TRN2", target_bir_lowering=False, debug=False)

    def din(name, shape, dt):
        return nc.dram_tensor(name, shape, dt, kind="ExternalInput").ap()

    cpk_d = din("cpk", [128, CKW], bf16)      # packed bf16 consts
    cps_d = din("cps", [32, 128], bf16)       # selb
    zlog_d = din("zlog", [32, V], bf16)
    cp8_d = din("cp8", [128, 256], f8)        # h0i | h1i
    cpf_d = din("cpf", [128, 256], f32)       # s0i | s1i
    w0e_d = din("w0e", [128, 4 * G], f8)
    eT_d = din("eT", [128, 4, NT * BL], f8)
    w0h_d = din("w0h", [128, 4 * G], f8)
    w1_d = din("w1", [128, 8 * G], f8)
    wout_d = din("wout", [128, 4 * V], f8)
    wta_d = din("wta", [128, 4 * COLS], f8)
    out_d = nc.dram_tensor("out_sd", [128, 32], f32,
                           kind="ExternalOutput").ap()

    with tile.TileContext(nc) as tc:
        from contextlib import ExitStack
        with ExitStack() as ctx:
            const = ctx.enter_context(tc.tile_pool(name="const", bufs=1))
            wpool = ctx.enter_context(tc.tile_pool(name="w", bufs=1))
            # startup-critical DMA order: w0e+eT head (first e-matmuls),
            # small const packs, w0h, then everything else
            w0e = wpool.tile([128, 2, 2, G], f8)
            nc.sync.dma_start(w0e[:], w0e_d[:])
            eT = wpool.tile([128, 2, 2, NT * BL], f8)
            # head: first 2 steps' embeddings so slot 0/1 e-matmuls can start
            # before the bulk lands
            nc.sync.dma_start(eT[:, :, :, 0:2 * BL], eT_d[:, :, 0:2 * BL])
            cpk = const.tile([128, CKW], bf16, tag="cpk")
            nc.sync.dma_start(cpk[:], cpk_d[:])
            cp8 = const.tile([128, 2, 4, 32], f8, tag="cp8")
            nc.sync.dma_start(cp8[:], cp8_d[:])
            cpf = const.tile([128, 256], f32, tag="cpf")
            nc.sync.dma_start(cpf[:], cpf_d[:])
            w0h = wpool.tile([128, 2, 2, G], f8)
            nc.sync.dma_start(w0h[:], w0h_d[:])
            nc.sync.dma_start(eT[:, :, :, 2 * BL:NT * BL],
                              eT_d[:, :, 2 * BL:NT * BL])
            w1 = wpool.tile([128, 4, 2, G], f8)
            nc.sync.dma_start(w1[:], w1_d[:])
            cps = const.tile([32, 128], bf16, tag="cps")
            nc.sync.dma_start(cps[:], cps_d[:])
            zlogt = const.tile([32, V], bf16, tag="zlogt")
            nc.sync.dma_start(zlogt[:], zlog_d[:])
            wout = wpool.tile([128, 2, 2, V], f8)
            nc.sync.dma_start(wout[:], wout_d[:])
            wta = wpool.tile([128, 4, COLS], f8)
            nc.sync.dma_start(wta[:], wta_d[:])

            onescol = cpk[:, CK_OC:CK_OC + 2]
            id128 = cpk[:, CK_ID:CK_ID + 128]
            zprM = cpk[:, CK_ZP:CK_ZP + 512]
            bg1M = cpk[:, CK_BG:CK_BG + 512]
            selb = cps[0:32, 0:128]
            zlog = zlogt
            h0i = cp8[:, 0, :, :]
            h1i = cp8[:, 1, :, :]
            s0i = cpf[:, 0:128]
            s1i = cpf[:, 128:256]

            state = ctx.enter_context(tc.tile_pool(name="state", bufs=1))
            HT = state.tile([128, 4, COLS], f8)
            sd_all = state.tile([128, 32], f32, tag="sd_all")
            sums_all = sd_all[:, 0:16]
            dps_all = sd_all[:, 16:32]
            nc.vector.memset(sd_all[:], 1.0)

            sact = ctx.enter_context(tc.tile_pool(name="sact", bufs=6))
            sdve = ctx.enter_context(tc.tile_pool(name="sdve", bufs=6))
            sst = ctx.enter_context(tc.tile_pool(name="sst", bufs=6))
            sexp = ctx.enter_context(tc.tile_pool(name="sexp", bufs=6))
            gsum = ctx.enter_context(tc.tile_pool(name="gsum", bufs=4))
            pvoc_cm = tc.tile_pool(name="pvoc", bufs=2, space="PSUM")
            pvoc = pvoc_cm.__enter__()

            # ---------------- vocab + target-dot pump ---------------------
            gsums = {}
            vwork = []
            vpushed = 0
            pending_dve = []

            def emit_vgroup(j, vi):
                base = 128 * j
                mj = min(128, COLS - base)
                vg0 = 1024 * vi
                vgs = min(1024, V - vg0)
                pl = pvoc.tile([128, 1024], f32, tag="pl")
                # DR moving operand free size is 2*vs -> keep vs <= 256.
                # pl spans TWO psum banks: zero region is per-bank, so the
                # first matmul touching EACH bank carries start=True and the
                # last one stop=True.
                for q in range(0, vgs, 256):
                    v0 = vg0 + q
                    vs = min(256, V - v0)
                    qs = slice(q, q + vs)
                    for g in range(2):
                        nc.tensor.matmul(
                            pl[:mj, qs], HT[:, 2 * g:2 * g + 2, base:base + mj],
                            wout[:, g, :, v0:v0 + vs],
                            start=(q % 512 == 0 and g == 0), stop=False,
                            perf_mode=DR, skip_group_check=True)
                for half in range(0, vgs, 512):
                    v0 = vg0 + half
                    vs = min(512, V - v0)
                    nc.tensor.matmul(pl[:mj, half:half + vs], selb[:, 0:mj],
                                     zlog[:, v0:v0 + vs],
                                     start=False, stop=True,
                                     skip_group_check=True)
                es = sexp.tile([128, 1024], bf16, tag="es")
                if vi == 0 and j < NTILE - 1:
                    gsums[j] = gsum.tile([128, 8], f32, tag="gs",
                                         name=f"gs{j}")
                # last tile: partials go straight into spare sums_all cols
                # (host sums them) -- drops the final reduce from the
                # end-of-program critical path
                tgt = (sums_all[:mj, 9 + vi:10 + vi] if j == NTILE - 1
                       else gsums[j][:mj, vi:vi + 1])
                if vi % 2 == 0:
                    nc.scalar.activation(es[:mj, 0:vgs], pl[:mj, 0:vgs],
                                         AF.Exp, accum_out=tgt)
                else:
                    # alternate: sum on DVE (saves ACT read-accum aux),
                    # deferred past the chain-critical u-chain STTs
                    et = es
                    pending_dve.append(
                        lambda tgt=tgt, et=et, mj=mj, vgs=vgs:
                        nc.vector.tensor_reduce(
                            tgt, et[:mj, 0:vgs],
                            mybir.AxisListType.XYZW, ALU.add))
                    nc.scalar.activation(es[:mj, 0:vgs], pl[:mj, 0:vgs],
                                         AF.Exp)
                if vi == NVG - 1 and j < NTILE - 1:
                    gt = gsums[j]
                    pending_dve.append(
                        lambda gt=gt, mj=mj, j=j:
                        nc.vector.tensor_reduce(
                            sums_all[:mj, j:j + 1], gt[:mj, 0:NVG],
                            mybir.AxisListType.XYZW, ALU.add))

            def emit_wta(j):
                base = 128 * j
                mj = min(128, COLS - base)
                dps = pdot.tile([128, 2], f32, tag="dps")
                for c in range(4):
                    sc = sexp.tile([128, 128], bf16, tag="sc")
                    nc.vector.tensor_mul(sc[:, 0:mj],
                                         HT[:, c, base:base + mj],
                                         wta[:, c, base:base + mj])
                    nc.tensor.matmul(dps[:mj, 0:2], sc[:, 0:mj],
                                     onescol[:, 0:2],
                                     start=(c == 0), stop=(c == 3))
                nc.vector.tensor_copy(dps_all[:mj, j:j + 1], dps[:mj, 0:1])

            def vocab_pump(t_done, n):
                nonlocal vpushed
                while vpushed < NTILE and min(4 * vpushed + 4, NT - 1) <= t_done:
                    for vi in range(NVG):
                        vwork.append((vpushed, vi))
                    vwork.append((vpushed, -1))
                    vpushed += 1
                for _ in range(n):
                    if not vwork:
                        return
                    j, vi = vwork.pop(0)
                    # low-priority band: the Tile scheduler treats vocab work
                    # as gap filler so it never blocks the recurrent chain
                    with tc.high_priority(offset=-VOFF):
                        if vi < 0:
                            emit_wta(j)
                        else:
                            emit_vgroup(j, vi)

            # ---------------- main recurrent loop --------------------------
            with tc.tile_pool(name="p0g", bufs=2, space="PSUM") as p0g, \
                 tc.tile_pool(name="p1g", bufs=1, space="PSUM") as p1g, \
                 tc.tile_pool(name="pdot", bufs=1, space="PSUM") as pdot:

                h0s_of = {-1: h0i}
                h1s_of = {-2: h1i, -1: h1i}
                s0_prev = s0i
                s1_prev = s1i

                for t in range(NT):
                    # l0 gate group: e-part + zpre (no recurrence deps)
                    g0 = p0g.tile([128, 512], f32, tag="g0")
                    for g in range(2):
                        for m in range(16):
                            ms = slice(32 * m, 32 * m + 32)
                            js = slice(128 * m, 128 * m + 128)
                            nc.tensor.matmul(
                                g0[:, ms], w0e[:, g, :, js],
                                eT[:, g, :, BL * t:BL * t + BL],
                                start=(m == 0 and g == 0), stop=False,
                                perf_mode=DR, skip_group_check=True)
                    nc.tensor.matmul(g0[:, 0:512], id128, zprM,
                                     start=False, stop=False,
                                     skip_group_check=True)

                    # l1(t-1) part A: h1(t-2) chunks + bias
                    if t > 0:
                        g1 = p1g.tile([128, 512], f32, tag="g1")
                        h1p = h1s_of[t - 2]
                        for m in range(16):
                            ms = slice(32 * m, 32 * m + 32)
                            js = slice(128 * m, 128 * m + 128)
                            for g in range(2):
                                nc.tensor.matmul(
                                    g1[:, ms], w1[:, g, :, js],
                                    h1p[:, 2 * g:2 * g + 2, :],
                                    start=(m == 0 and g == 0), stop=False,
                                    perf_mode=DR, skip_group_check=True)
                        nc.tensor.matmul(g1[:, 0:512], id128, bg1M,
                                         start=False, stop=False,
                                         skip_group_check=True)

                    # vocab filler: matmuls early so exp input is ready
                    if t >= VOC_START:
                        vocab_pump(t - 2, PRE_PACE)

                    # l0 h-part (closes g0) -- needs h0s(t-1)
                    h0p = h0s_of[t - 1]
                    for m in range(16):
                        ms = slice(32 * m, 32 * m + 32)
                        js = slice(128 * m, 128 * m + 128)
                        for g in range(2):
                            nc.tensor.matmul(
                                g0[:, ms], w0h[:, g, :, js],
                                h0p[:, 2 * g:2 * g + 2, :],
                                start=False,
                                stop=(m == 15 and g == 1),
                                perf_mode=DR, skip_group_check=True)

                    # l1(t-1) part B: h0(t-1) chunks (closes g1)
                    if t > 0:
                        for m in range(16):
                            ms = slice(32 * m, 32 * m + 32)
                            js = slice(128 * m, 128 * m + 128)
                            for g in range(2):
                                nc.tensor.matmul(
                                    g1[:, ms], w1[:, 2 + g, :, js],
                                    h0p[:, 2 * g:2 * g + 2, :],
                                    start=False,
                                    stop=(m == 15 and g == 1),
                                    perf_mode=DR, skip_group_check=True)

                    # ---- elementwise tails. ACT order: tA0, tA1, th0, th1
                    s01 = sst.tile([128, 256], f32, tag="s01")
                    tA0 = sact.tile([128, 512], bf16, tag="a0", name="tA0")
                    nc.scalar.activation(tA0[:], g0[:], AF.Tanh)
                    if t > 0:
                        tA1 = sact.tile([128, 512], bf16, tag="a1",
                                        name="tA1")
                        with tc.high_priority(offset=-L1OFF):
                            nc.scalar.activation(tA1[:], g1[:], AF.Tanh)
                    # DVE: layer-0 u-chain
                    u10 = sdve.tile([128, 128], f32, tag="u10", name="u10")
                    nc.vector.scalar_tensor_tensor(
                        u10[:], tA0[:, 0:128], 1.0, s0_prev, ALU.add, ALU.mult)
                    u20 = sdve.tile([128, 128], f32, tag="u20", name="u20")
                    nc.vector.scalar_tensor_tensor(
                        u20[:], tA0[:, 128:256], 1.0, tA0[:, 256:384],
                        ALU.add, ALU.mult)
                    nc.vector.scalar_tensor_tensor(
                        s01[:, 0:128], u10[:], 0.5, u20[:], ALU.mult, ALU.add)
                    th01 = sdve.tile([128, 256], bf16, tag="th01")
                    nc.scalar.activation(th01[:, 0:128], s01[:, 0:128],
                                         AF.Tanh, scale=0.5)
                    s0_prev = s01[:, 0:128]
                    if t > 0:
                        # layer-1 u-chain start (fits before th0 lands)
                        u11 = sdve.tile([128, 128], f32, tag="u11",
                                        name="u11")
                        with tc.high_priority(offset=-L1OFF):
                            nc.vector.scalar_tensor_tensor(
                                u11[:], tA1[:, 0:128], 1.0, s1_prev,
                                ALU.add, ALU.mult)
                    # h0 chain: hsn0 right after th0
                    hsn0 = sst.tile([128, 4, 32], f8, tag="h0s", name="hsn0")
                    nc.vector.scalar_tensor_tensor(
                        hsn0[:], tA0[:, 384:512], 1.0, th01[:, 0:128],
                        ALU.add, ALU.mult)
                    h0s_of[t] = hsn0
                    if t > 0:
                        u21 = sdve.tile([128, 128], f32, tag="u21",
                                        name="u21")
                        hsn1 = sst.tile([128, 4, 32], f8, tag="h1s",
                                        name="hsn1")
                        with tc.high_priority(offset=-L1OFF):
                            nc.vector.scalar_tensor_tensor(
                                u21[:], tA1[:, 128:256], 1.0, tA1[:, 256:384],
                                ALU.add, ALU.mult)
                            nc.vector.scalar_tensor_tensor(
                                s01[:, 128:256], u11[:], 0.5, u21[:],
                                ALU.mult, ALU.add)
                            nc.scalar.activation(th01[:, 128:256],
                                                 s01[:, 128:256],
                                                 AF.Tanh, scale=0.5)
                            nc.vector.scalar_tensor_tensor(
                                hsn1[:], tA1[:, 384:512], 1.0,
                                th01[:, 128:256], ALU.add, ALU.mult)
                            tw = t - 1
                            for c in range(4):
                                nc.gpsimd.tensor_add(
                                    HT[:, c, 32 * tw:32 * tw + 32],
                                    h0s_of[tw][:, c, :], hsn1[:, c, :])
                        h1s_of[t - 1] = hsn1
                        s1_prev = s01[:, 128:256]
                        del h1s_of[t - 3]
                        del h0s_of[t - 2]
                    # vocab filler: exp lands after th1 in the ACT queue
                    if t >= VOC_START:
                        vocab_pump(t - 2, POST_PACE)
                    with tc.high_priority(offset=-VOFF):
                        for fn in pending_dve:
                            fn()
                    pending_dve.clear()

                # flush l1(NT-1)
                t = NT
                g1 = p1g.tile([128, 512], f32, tag="g1")
                h1p = h1s_of[t - 2]
                h0p = h0s_of[t - 1]
                for m in range(16):
                    ms = slice(32 * m, 32 * m + 32)
                    js = slice(128 * m, 128 * m + 128)
                    for g in range(2):
                        nc.tensor.matmul(g1[:, ms], w1[:, g, :, js],
                                         h1p[:, 2 * g:2 * g + 2, :],
                                         start=(m == 0 and g == 0), stop=False,
                                         perf_mode=DR, skip_group_check=True)
                nc.tensor.matmul(g1[:, 0:512], id128, bg1M,
                                 start=False, stop=False,
                                 skip_group_check=True)
                for m in range(16):
                    ms = slice(32 * m, 32 * m + 32)
                    js = slice(128 * m, 128 * m + 128)
                    for g in range(2):
                        nc.tensor.matmul(g1[:, ms], w1[:, 2 + g, :, js],
                                         h0p[:, 2 * g:2 * g + 2, :],
                                         start=False,
                                         stop=(m == 15 and g == 1),
                                         perf_mode=DR, skip_group_check=True)

                s01f = sst.tile([128, 256], f32, tag="s01")
                tA1f = sact.tile([128, 512], bf16, tag="a1", name="tA1f")
                nc.scalar.activation(tA1f[:], g1[:], AF.Tanh)
                u11f = sdve.tile([128, 128], f32, tag="u11", name="u11f")
                nc.vector.scalar_tensor_tensor(
                    u11f[:], tA1f[:, 0:128], 1.0, s1_prev, ALU.add, ALU.mult)
                u21f = sdve.tile([128, 128], f32, tag="u21", name="u21f")
                nc.vector.scalar_tensor_tensor(
                    u21f[:], tA1f[:, 128:256], 1.0, tA1f[:, 256:384],
                    ALU.add, ALU.mult)
                nc.vector.scalar_tensor_tensor(
                    s01f[:, 128:256], u11f[:], 0.5, u21f[:],
                    ALU.mult, ALU.add)
                th01f = sdve.tile([128, 256], bf16, tag="th01")
                nc.scalar.activation(th01f[:, 128:256], s01f[:, 128:256],
                                     AF.Tanh, scale=0.5)
                hsn1f = sst.tile([128, 4, 32], f8, tag="h1s", name="hsn1f")
                nc.vector.scalar_tensor_tensor(
                    hsn1f[:], tA1f[:, 384:512], 1.0, th01f[:, 128:256],
                    ALU.add, ALU.mult)
                tw = NT - 1
                for c in range(4):
                    nc.gpsimd.tensor_add(
                        HT[:, c, 32 * tw:32 * tw + 32],
                        h0s_of[tw][:, c, :], hsn1f[:, c, :])

                # drain remaining vocab + wta work, then finalize
                vocab_pump(NT - 1, len(vwork) + NVG + 2)
                for fn in pending_dve:
                    fn()
                pending_dve.clear()
                nc.sync.dma_start(out_d[:, :], sd_all[:, :])
            pvoc_cm.__exit__(None, None, None)

    nc.compile()
    return nc


def _prep_host(inputs):
    z = np.asarray(inputs["z"], np.float32)
    x = np.asarray(inputs["x"])
    emb = np.asarray(inputs["emb"], np.float32)
    Wg0 = np.asarray(inputs["Wg0"], np.float32)
    bg0 = np.asarray(inputs["bg0"], np.float32)
    Wg1 = np.asarray(inputs["Wg1"], np.float32)
    bg1 = np.asarray(inputs["bg1"], np.float32)
    Wout = np.asarray(inputs["Wout"], np.float32)
    bout = np.asarray(inputs["bout"], np.float32)
    tw1 = np.asarray(inputs["tw1"], np.float32)
    tb1 = np.asarray(inputs["tb1"], np.float32)
    tw2 = np.asarray(inputs["tw2"], np.float32)
    tb2 = np.asarray(inputs["tb2"], np.float32)

    bf = ml_dtypes.bfloat16
    f8h = ml_dtypes.float8_e4m3fn

    def permute_rows(W):
        # (i, f, o, cn) blocks -> (f, i, cn, o)
        return np.concatenate(
            [W[512:1024], W[0:512], W[1536:2048], W[1024:1536]], axis=0)

    rs = np.repeat([0.5, 0.5, 1.0, 0.5], 512).astype(np.float32)[:, None]

    def chunked(a, nch):
        # [128*nch, N] -> [128, nch*N] (chunk-major free layout)
        n = a.shape[1]
        return np.ascontiguousarray(
            a.reshape(nch, 128, n).transpose(1, 0, 2).reshape(128, nch * n))

    W0 = Wg0.reshape(G, D + Z + D)
    W0p = permute_rows(W0) * rs
    bg0p = (permute_rows(bg0.reshape(G, 1)) * rs)[:, 0]
    W1 = Wg1.reshape(G, 2 * D)
    W1p = permute_rows(W1) * rs * 0.5
    bg1p = (permute_rows(bg1.reshape(G, 1)) * rs)[:, 0]
    W0z_s = W0p[:, 1024:1152]

    shared = {
        # chunked() already yields the [q, chunk, ...] layout; DoubleRow just
        # reinterprets chunk index as (pair g, ktile p)
        "w0h": chunked(W0p[:, 0:512].T * 0.5, 4).astype(f8h),
        "w0e": chunked(W0p[:, 512:1024].T, 4).astype(f8h),
        "w1": chunked(W1p.T, 8).astype(f8h),
        "wout": chunked(Wout[:, 0:512].T * 0.5, 4).astype(f8h),
    }

    def injM(ZP):
        # [32 b, 2048 gate] -> [128 q, 512 (m,b)] moving layout:
        # injM[q, 32m+b] = ZP[b, 128m+q]
        return np.ascontiguousarray(
            ZP.T.reshape(16, 128, 32).transpose(1, 0, 2).reshape(128, 512))

    cps0 = np.tile(np.eye(32, dtype=bf), (1, 4))  # selb [32, 128]

    cpk_base = np.zeros((128, CKW), bf)
    cpk_base[:, CK_OC:CK_OC + 2] = 1.0
    cpk_base[:, CK_ID:CK_ID + 128] = np.eye(128, dtype=bf)
    cpk_base[:, CK_BG:CK_BG + 512] = injM(
        np.broadcast_to(bg1p, (32, G))).astype(bf)

    def packT(a):  # [32, 512] -> [128, 128] transposed chunk-packed
        return np.ascontiguousarray(
            a.T.reshape(4, 128, 32).transpose(1, 0, 2).reshape(128, 128))

    in_maps = []
    extra = []
    for cidx in range(NC):
        bs = slice(BL * cidx, BL * cidx + BL)
        z_c = z[bs]
        x_c = x[bs]
        xn = x_c[:, 1:T]

        m = dict(shared)
        cp8 = np.zeros((128, 256), f8h)
        cpf = np.zeros((128, 256), np.float32)
        for l in range(2):
            u = np.maximum(z_c @ tw1[l].T + tb1[l], 0.0)
            hh = np.tanh(u @ tw2[l].T + tb2[l])
            cp8[:, 128 * l:128 * l + 128] = packT(
                2.0 * hh[:, 0:512]).astype(f8h)
            cpf[:, 128 * l:128 * l + 128] = packT(2.0 * hh[:, 512:1024])
        m["cp8"] = cp8
        m["cpf"] = cpf

        cpk = cpk_base.copy()
        cpk[:, CK_ZP:CK_ZP + 512] = injM(
            (z_c @ W0z_s.T + bg0p).astype(np.float32)).astype(bf)
        m["cpk"] = cpk
        m["cps"] = cps0
        zlog_f = z_c @ Wout[:, 512:640].T + bout
        m["zlog"] = zlog_f.astype(bf)
        tdz = np.take_along_axis(zlog_f, xn, axis=1)
        extra.append(tdz.sum(axis=1))

        embx = emb[x_c[:, 0:NT]]
        m["eT"] = np.ascontiguousarray(
            embx.transpose(2, 1, 0).reshape(4, 128, NT * BL)
            .transpose(1, 0, 2).reshape(128, 4 * NT * BL)).astype(f8h)
        wrows = Wout[xn][:, :, 0:512] * 0.5
        m["wta"] = np.ascontiguousarray(
            wrows.transpose(2, 1, 0).reshape(4, 128, COLS)
            .transpose(1, 0, 2).reshape(128, 4 * COLS)).astype(f8h)
        in_maps.append(m)
    return in_maps, extra


def kernel(**inputs) -> np.ndarray:
    if "nc" not in _CACHE:
        _CACHE["nc"] = _build()
    nc = _CACHE["nc"]
    in_maps, extra = _prep_host(inputs)
    res = bass_utils.run_bass_kernel_spmd(nc, in_maps, core_ids=list(range(NC)))
    out = np.zeros((B, 1), np.float32)
    for cidx in range(NC):
        sd = np.array(res.results[cidx]["out_sd"])   # [128, 32]
        sd[:, NTILE - 1] = sd[:, 9:14].sum(axis=1)   # fold last tile partials
        lpc = sd[:, 16:16 + NTILE] - np.log(sd[:, 0:NTILE])
        lp = lpc.T.reshape(-1)[:COLS].reshape(NT, BL)
        out[BL * cidx:BL * cidx + BL, 0] = lp.sum(axis=0) + extra[cidx]
    return out
